# revision 2
# baseline (speedup 1.0000x reference)
"""Causal attention (B=8, S=2048, D=768, single head) on 8 trn2 NeuronCores.

Sharding: data-parallel over batch — core b computes batch element b.

Host-side prep (layout only): x is passed transposed per core (xt = x[b].T,
[D, S]) and the Q/K weights are passed transposed (wqt = wq.T, wkt = wk.T),
so the kernel needs no PE transposes at all.

Algorithm per core:
  0. A = Wq @ Wk^T  [d, d'] computed once on device:
     A_blk = matmul(lhsT=wqt[e, d-blk], rhs=wkt[e, d']) contracting e.
     Then scores = x A x^T, so the K projection disappears:
     scoresT tile [k, q] = matmul(lhsT=xT[d, k-blk], rhs=Q'T[d, q]) with
     Q'T = matmul(lhsT=A[d, d'-blk], rhs=xT[d, s]) (Q' = x @ A).
  Per 512-wide q-chunk:
  1. Q'T chunk [d', q] (contract d over 6 blocks)
  2. V chunk [s, e] natural (lhsT = xT s-block, rhs = wv) + ones cols,
     stored bf16
  3. scoresT tiles [k-blk, q] contracting d; exp (scale fused) on ACT ->
     P^T tiles in bf16; causal mask multiply on partial diagonal blocks
  4. out[q, e] (+ denominator via V ones cols) = PT_blk.T @ V_blk
     (contract k, bf16 operands)
  5. normalize by reciprocal of denominator column, DMA out

Matmul operands are float32r (full-rate PE) except the P·V stage which is
bf16 (same rate, no >=256 free-dim constraint, half SBUF).
"""

import os

import numpy as np

import concourse.bass as bass
import concourse.mybir as mybir
from concourse import bacc
from concourse.tile import TileContext
from concourse.bass_utils import run_bass_kernel_spmd

B, S, D = 8, 2048, 768
P = 128
ND = D // P            # 6 feature blocks
NB = S // P            # 16 seq blocks
CH = 512               # s-chunk width
NCH = S // CH          # 4 chunks
QPC = CH // P          # 4 q-blocks per chunk
SCALE = 1.0 / float(np.sqrt(D))
F32 = mybir.dt.float32
BF16 = mybir.dt.bfloat16

MM_MODE = os.environ.get("KMM", "f32r")  # f32r | fp32
MDT = mybir.dt.float32r if MM_MODE == "f32r" else F32


def _build_nc():
    nc = bacc.Bacc(None, target_bir_lowering=False)
    xt_d = nc.dram_tensor("xt", [D, S], F32, kind="ExternalInput")
    wqt_d = nc.dram_tensor("wqt", [D, D], F32, kind="ExternalInput")
    wkt_d = nc.dram_tensor("wkt", [D, D], F32, kind="ExternalInput")
    wv_d = nc.dram_tensor("wv", [D, D], F32, kind="ExternalInput")
    out_d = nc.dram_tensor("out", [S, D], F32, kind="ExternalOutput")

    xt_r = xt_d[:, :].rearrange("(o p) s -> p o s", p=P).bitcast(MDT)
    wqt_r = wqt_d[:, :].rearrange("(o p) d -> p o d", p=P).bitcast(MDT)
    wkt_r = wkt_d[:, :].rearrange("(o p) d -> p o d", p=P).bitcast(MDT)
    wv_r = wv_d[:, :].rearrange("(o p) e -> p o e", p=P).bitcast(MDT)

    with TileContext(nc) as tc:
        with (
            tc.tile_pool(name="const", bufs=1) as constp,
            tc.tile_pool(name="persist", bufs=1) as persist,
            tc.tile_pool(name="qt", bufs=1) as qtp,
            tc.tile_pool(name="pt", bufs=1) as ptp,
            tc.tile_pool(name="outp", bufs=2) as outp,
            tc.tile_pool(name="rc", bufs=4) as rcp,
            tc.tile_pool(name="psA", bufs=2, space="PSUM") as psA,
            tc.tile_pool(name="psQKV", bufs=2, space="PSUM") as psQKV,
            tc.tile_pool(name="psO", bufs=4, space="PSUM") as psO,
        ):
            # smask[p, g] = 1.0 if g >= p + 128 else 0.0 (bf16); slices give
            # the partial-diagonal causal masks for P^T tiles.
            smask_f = constp.tile([P, 640], F32)
            nc.gpsimd.memset(smask_f, 1.0)
            nc.gpsimd.affine_select(
                out=smask_f,
                in_=smask_f,
                compare_op=mybir.AluOpType.is_ge,
                fill=0.0,
                base=-128,
                pattern=[[1, 640]],
                channel_multiplier=-1,
            )
            smask = constp.tile([P, 640], BF16)
            nc.vector.tensor_copy(smask, smask_f)

            XT = persist.tile([P, ND, S], MDT)       # x^T: [d_in, do, s]
            V = persist.tile([P, NB, D + 2], BF16)   # [s_in, sb, e]; cols D,D+1 = 1.0
            WV = persist.tile([P, ND, D], MDT)       # wv: [d_in, do, e]
            A_sb = persist.tile([P, ND, D], MDT)     # A=wq@wk.T: [d_in, do, d']
            WQT = persist.tile([P, ND, D], MDT)      # wq^T: [e_in, eo, d]
            WKT = persist.tile([P, ND, D], MDT)      # wk^T: [e_in, eo, d']
            ones_col = constp.tile([P, NB, 2], BF16)
            nc.vector.memset(ones_col, 1.0)
            nc.vector.tensor_copy(V[:, :, D : D + 2], ones_col)

            h3 = ND // 2

            # ---- prologue DMAs (first-needed first)
            nc.sync.dma_start(WV[:, 0:h3], wv_r[:, 0:h3])
            nc.sync.dma_start(WV[:, h3:ND], wv_r[:, h3:ND])
            for do in range(ND):
                nc.scalar.dma_start(
                    XT[:, do, 0:CH], xt_r[:, do, 0:CH]
                )
            nc.gpsimd.dma_start(WQT[:, 0:h3], wqt_r[:, 0:h3])
            nc.gpsimd.dma_start(WQT[:, h3:ND], wqt_r[:, h3:ND])
            nc.gpsimd.dma_start(WKT[:, 0:h3], wkt_r[:, 0:h3])
            nc.gpsimd.dma_start(WKT[:, h3:ND], wkt_r[:, h3:ND])

            pt_tiles = {}

            def emit_v_chunk(c):
                for sb4 in range(QPC):
                    sb = c * QPC + sb4
                    s0 = sb * P
                    pv0 = psQKV.tile([P, CH], F32, tag="qkv")
                    for do in range(ND):
                        nc.tensor.matmul(
                            pv0,
                            XT[:, do, s0 : s0 + P],
                            WV[:, do, 0:CH],
                            start=(do == 0),
                            stop=(do == ND - 1),
                        )
                    nc.scalar.copy(V[:, sb, 0:CH], pv0)
                    pv1 = psQKV.tile([P, CH], F32, tag="qkv")
                    for do in range(ND):
                        nc.tensor.matmul(
                            pv1[:, 0 : D - CH],
                            XT[:, do, s0 : s0 + P],
                            WV[:, do, CH:D],
                            start=(do == 0),
                            stop=(do == ND - 1),
                        )
                    nc.scalar.copy(V[:, sb, CH:D], pv1[:, 0 : D - CH])

            # V(chunk 0) is the PE warmup work while wqt/wkt stream in.
            emit_v_chunk(0)

            # ---- A = Wq @ Wk^T, contracting e
            for db in range(ND):
                pa0 = psQKV.tile([P, CH], F32, tag="qkv")
                for eo in range(ND):
                    nc.tensor.matmul(
                        pa0,
                        WQT[:, eo, db * P : (db + 1) * P],
                        WKT[:, eo, 0:CH],
                        start=(eo == 0),
                        stop=(eo == ND - 1),
                    )
                nc.vector.tensor_copy(A_sb[:, db, 0:CH], pa0)
                pa1 = psQKV.tile([P, CH], F32, tag="qkv")
                for eo in range(ND):
                    nc.tensor.matmul(
                        pa1[:, 0 : D - CH],
                        WQT[:, eo, db * P : (db + 1) * P],
                        WKT[:, eo, CH:D],
                        start=(eo == 0),
                        stop=(eo == ND - 1),
                    )
                nc.vector.tensor_copy(A_sb[:, db, CH:D], pa1[:, 0 : D - CH])

            for c in range(NCH):
                # ---- prefetch next x chunk
                if c + 1 < NCH:
                    c0 = (c + 1) * CH
                    for do in range(ND):
                        eng = nc.sync if do % 2 == 0 else nc.gpsimd
                        eng.dma_start(
                            XT[:, do, c0 : c0 + CH], xt_r[:, do, c0 : c0 + CH]
                        )

                # ---- Q'T for this chunk: [d'-blk, q], contract d
                QTc = qtp.tile([P, ND, CH], MDT, tag="qt")
                for eb in range(ND):
                    pq = psQKV.tile([P, CH], F32, tag="qkv")
                    for do in range(ND):
                        nc.tensor.matmul(
                            pq,
                            A_sb[:, do, eb * P : (eb + 1) * P],
                            XT[:, do, c * CH : (c + 1) * CH],
                            start=(do == 0),
                            stop=(do == ND - 1),
                        )
                    nc.vector.tensor_copy(QTc[:, eb, :], pq)

                # ---- V for this chunk (chunk 0 emitted in prologue)
                if c > 0:
                    emit_v_chunk(c)

                # ---- scores^T + exp (+ causal mask on partial blocks)
                # For diagonal blocks (kb = 4c+i, i>0) only q-cols >= i*128
                # are causally live and AV never reads the dead columns, so
                # narrow the matmul/exp/mask to the live width (min 256 to
                # stay on the f32r full-rate path).
                nkb = QPC * (c + 1)
                for kb in range(nkb):
                    i = kb - QPC * c
                    q0 = max(i, 0) * P
                    if CH - q0 < 256:
                        q0 = CH - 256
                    W = CH - q0
                    ps_s = psA.tile([P, CH], F32, tag="a")
                    for eo in range(ND):
                        nc.tensor.matmul(
                            ps_s[:, 0:W],
                            XT[:, eo, kb * P : (kb + 1) * P],
                            QTc[:, eo, q0:CH],
                            start=(eo == 0),
                            stop=(eo == ND - 1),
                        )
                    ptw = {13: 384, 14: 256, 15: 256}.get(kb, CH)
                    base = CH - ptw
                    pt = ptp.tile([P, ptw], BF16, tag=f"pt{kb}")
                    nc.scalar.activation(
                        pt[:, q0 - base : CH - base],
                        ps_s[:, 0:W],
                        mybir.ActivationFunctionType.Exp,
                        scale=SCALE,
                    )
                    pt_tiles[kb] = (pt, base)
                    if kb >= QPC * c:
                        off = c * CH - kb * P + 384
                        nc.vector.tensor_mul(
                            pt[:, q0 - base : CH - base],
                            pt[:, q0 - base : CH - base],
                            smask[:, off + q0 - 256 : off + CH - 256],
                        )

                # ---- attn @ [V | 1], normalize, store
                for qs in range(QPC):
                    qb = c * QPC + qs
                    po0 = psO.tile([P, CH], F32, tag="o")
                    po1 = psO.tile([P, CH], F32, tag="o")
                    for kb in range(qb + 1):
                        ptk, pbase = pt_tiles[kb]
                        lhs = ptk[:, qs * P - pbase : (qs + 1) * P - pbase]
                        nc.tensor.matmul(
                            po0,
                            lhs,
                            V[:, kb, 0:CH],
                            start=(kb == 0),
                            stop=(kb == qb),
                        )
                        nc.tensor.matmul(
                            po1[:, 0 : D + 2 - CH],
                            lhs,
                            V[:, kb, CH : D + 2],
                            start=(kb == 0),
                            stop=(kb == qb),
                        )
                    recip = rcp.tile([P, 1], F32, tag="rc")
                    nc.vector.reciprocal(recip, po1[:, D - CH : D - CH + 1])
                    o_sb = outp.tile([P, D], F32, tag="o")
                    nc.vector.tensor_scalar_mul(o_sb[:, 0:CH], po0, recip)
                    nc.vector.tensor_scalar_mul(
                        o_sb[:, CH:D], po1[:, 0 : D - CH], recip
                    )
                    nc.scalar.dma_start(out_d[qb * P : (qb + 1) * P, :], o_sb)

    nc.finalize()
    return nc


_NC_CACHE = None


def _get_nc():
    global _NC_CACHE
    if _NC_CACHE is None:
        _NC_CACHE = _build_nc()
    return _NC_CACHE


def run(inputs, trace=False):
    x = np.asarray(inputs["x"], dtype=np.float32)
    wq = np.asarray(inputs["wq"], dtype=np.float32)
    wk = np.asarray(inputs["wk"], dtype=np.float32)
    wv = np.asarray(inputs["wv"], dtype=np.float32)
    nc = _get_nc()
    wqt = np.ascontiguousarray(wq.T)
    wkt = np.ascontiguousarray(wk.T)
    in_maps = [
        {
            "xt": np.ascontiguousarray(x[b].T),
            "wqt": wqt,
            "wkt": wkt,
            "wv": wv,
        }
        for b in range(B)
    ]
    res = run_bass_kernel_spmd(nc, in_maps, core_ids=list(range(B)), trace=trace)
    out = np.stack([r["out"] for r in res.results]).astype(np.float32)
    return out, res


def kernel(x, wq, wk, wv):
    out, _ = run({"x": x, "wq": wq, "wk": wk, "wv": wv}, trace=False)
    return out


# revision 8
# speedup vs baseline: 1.2220x; 1.2220x over previous
"""Causal attention (B=8, S=2048, D=768, single head) on 8 trn2 NeuronCores.

Sharding: data-parallel over batch — core b computes batch element b.

Host-side prep (layout only): x is passed transposed per core (xt = x[b].T,
[D, S]) and the Q/K weights are passed transposed (wqt = wq.T, wkt = wk.T),
so the kernel needs no PE transposes at all.

Algorithm per core:
  0. A = Wq @ Wk^T  [d, d'] computed once on device:
     A_blk = matmul(lhsT=wqt[e, d-blk], rhs=wkt[e, d']) contracting e.
     Then scores = x A x^T, so the K projection disappears:
     scoresT tile [k, q] = matmul(lhsT=xT[d, k-blk], rhs=Q'T[d, q]) with
     Q'T = matmul(lhsT=A[d, d'-blk], rhs=xT[d, s]) (Q' = x @ A).
  Per 512-wide q-chunk:
  1. Q'T chunk [d', q] (contract d over 6 blocks)
  2. V chunk [s, e] natural (lhsT = xT s-block, rhs = wv) + ones cols,
     stored bf16
  3. scoresT tiles [k-blk, q] contracting d; exp (scale fused) on ACT ->
     P^T tiles in bf16; causal mask multiply on partial diagonal blocks
  4. out[q, e] (+ denominator via V ones cols) = PT_blk.T @ V_blk
     (contract k, bf16 operands)
  5. normalize by reciprocal of denominator column, DMA out

Matmul operands are float32r (full-rate PE) except the P·V stage which is
bf16 (same rate, no >=256 free-dim constraint, half SBUF).
"""

import os

import numpy as np

import concourse.bass as bass
import concourse.mybir as mybir
from concourse import bacc
from concourse.tile import TileContext
from concourse.bass_utils import run_bass_kernel_spmd

B, S, D = 8, 2048, 768
P = 128
ND = D // P            # 6 feature blocks
NB = S // P            # 16 seq blocks
CH = 512               # s-chunk width
NCH = S // CH          # 4 chunks
QPC = CH // P          # 4 q-blocks per chunk
SCALE = 1.0 / float(np.sqrt(D))
F32 = mybir.dt.float32
BF16 = mybir.dt.bfloat16

MM_MODE = os.environ.get("KMM", "f32r")  # f32r | fp32
MDT = mybir.dt.float32r if MM_MODE == "f32r" else F32


def _build_nc():
    nc = bacc.Bacc(None, target_bir_lowering=False)
    xt_d = nc.dram_tensor("xt", [D, S], F32, kind="ExternalInput")
    wqt_d = nc.dram_tensor("wqt", [D, D], F32, kind="ExternalInput")
    wkt_d = nc.dram_tensor("wkt", [D, D], F32, kind="ExternalInput")
    wv_d = nc.dram_tensor("wv", [D, D], F32, kind="ExternalInput")
    out_d = nc.dram_tensor("out", [S, D], F32, kind="ExternalOutput")

    xt_r = xt_d[:, :].rearrange("(o p) s -> p o s", p=P).bitcast(MDT)
    wqt_r = wqt_d[:, :].rearrange("(o p) d -> p o d", p=P).bitcast(MDT)
    wkt_r = wkt_d[:, :].rearrange("(o p) d -> p o d", p=P).bitcast(MDT)
    wv_r = wv_d[:, :].rearrange("(o p) e -> p o e", p=P).bitcast(MDT)

    with TileContext(nc) as tc:
        with (
            tc.tile_pool(name="const", bufs=1) as constp,
            tc.tile_pool(name="persist", bufs=1) as persist,
            tc.tile_pool(name="qt", bufs=1) as qtp,
            tc.tile_pool(name="pt", bufs=1) as ptp,
            tc.tile_pool(name="outp", bufs=2) as outp,
            tc.tile_pool(name="rc", bufs=4) as rcp,
            tc.tile_pool(name="psA", bufs=2, space="PSUM") as psA,
            tc.tile_pool(name="psQKV", bufs=2, space="PSUM") as psQKV,
            tc.tile_pool(name="psO", bufs=4, space="PSUM") as psO,
        ):
            # smask[p, g] = 1.0 if g >= p + 128 else 0.0 (bf16); slices give
            # the partial-diagonal causal masks for P^T tiles.
            smask_f = constp.tile([P, 640], F32)
            nc.gpsimd.memset(smask_f, 1.0)
            nc.gpsimd.affine_select(
                out=smask_f,
                in_=smask_f,
                compare_op=mybir.AluOpType.is_ge,
                fill=0.0,
                base=-128,
                pattern=[[1, 640]],
                channel_multiplier=-1,
            )
            smask = constp.tile([P, 640], BF16)
            nc.vector.tensor_copy(smask, smask_f)

            XT = persist.tile([P, ND, S], MDT)       # x^T: [d_in, do, s]
            V = persist.tile([P, NB, D + 2], BF16)   # [s_in, sb, e]; cols D,D+1 = 1.0
            WV = persist.tile([P, ND, D], MDT)       # wv: [d_in, do, e]
            A_sb = persist.tile([P, ND, D], MDT)     # A=wq@wk.T: [d_in, do, d']
            WQT = persist.tile([P, ND, D], MDT)      # wq^T: [e_in, eo, d]
            WKT = persist.tile([P, ND, D], MDT)      # wk^T: [e_in, eo, d']
            ones_col = constp.tile([P, NB, 2], F32)
            nc.vector.memset(ones_col, 1.0)
            nc.vector.tensor_copy(V[:, :, D : D + 2], ones_col)

            h3 = ND // 2

            # ---- prologue DMAs (first-needed first, spread over queues)
            nc.sync.dma_start(WV[:, 0:h3], wv_r[:, 0:h3])
            nc.sync.dma_start(WV[:, h3:ND], wv_r[:, h3:ND])
            for do in range(ND):
                nc.scalar.dma_start(
                    XT[:, do, 0:CH], xt_r[:, do, 0:CH]
                )
            nc.gpsimd.dma_start(WQT[:, 0:h3], wqt_r[:, 0:h3])
            nc.gpsimd.dma_start(WQT[:, h3:ND], wqt_r[:, h3:ND])
            nc.gpsimd.dma_start(WKT[:, 0:h3], wkt_r[:, 0:h3])
            nc.gpsimd.dma_start(WKT[:, h3:ND], wkt_r[:, h3:ND])

            # ---- PE warmup: dense dummy matmuls while prologue DMAs land.
            # Gets the HAM clock gate to K=8/8 (~3.4us of sustained PE
            # activity) before the first real matmul, instead of paying the
            # half-rate ramp on real work.
            warm_f = constp.tile([P, CH], F32)
            nc.vector.memset(warm_f, 0.0)
            warm = constp.tile([P, CH], MDT)
            nc.vector.tensor_copy(warm, warm_f)
            for w in range(28):
                pw = psA.tile([P, CH], F32, tag="a")
                nc.tensor.matmul(
                    pw, warm[:, 0:P], warm, start=True, stop=True
                )

            pt_tiles = {}

            def emit_v_chunk(c):
                for sb4 in range(QPC):
                    sb = c * QPC + sb4
                    s0 = sb * P
                    pv0 = psQKV.tile([P, CH], F32, tag="qkv")
                    for do in range(ND):
                        nc.tensor.matmul(
                            pv0,
                            XT[:, do, s0 : s0 + P],
                            WV[:, do, 0:CH],
                            start=(do == 0),
                            stop=(do == ND - 1),
                        )
                    nc.scalar.copy(V[:, sb, 0:CH], pv0)
                    pv1 = psQKV.tile([P, CH], F32, tag="qkv")
                    for do in range(ND):
                        nc.tensor.matmul(
                            pv1[:, 0 : D - CH],
                            XT[:, do, s0 : s0 + P],
                            WV[:, do, CH:D],
                            start=(do == 0),
                            stop=(do == ND - 1),
                        )
                    nc.scalar.copy(V[:, sb, CH:D], pv1[:, 0 : D - CH])

            # V(chunk 0) is the PE warmup work while wqt/wkt stream in.
            emit_v_chunk(0)

            # ---- A = Wq @ Wk^T, contracting e
            for db in range(ND):
                pa0 = psQKV.tile([P, CH], F32, tag="qkv")
                for eo in range(ND):
                    nc.tensor.matmul(
                        pa0,
                        WQT[:, eo, db * P : (db + 1) * P],
                        WKT[:, eo, 0:CH],
                        start=(eo == 0),
                        stop=(eo == ND - 1),
                    )
                nc.vector.tensor_copy(A_sb[:, db, 0:CH], pa0)
                pa1 = psQKV.tile([P, CH], F32, tag="qkv")
                for eo in range(ND):
                    nc.tensor.matmul(
                        pa1[:, 0 : D - CH],
                        WQT[:, eo, db * P : (db + 1) * P],
                        WKT[:, eo, CH:D],
                        start=(eo == 0),
                        stop=(eo == ND - 1),
                    )
                nc.vector.tensor_copy(A_sb[:, db, CH:D], pa1[:, 0 : D - CH])

            for c in range(NCH):
                # ---- prefetch next x chunk
                if c + 1 < NCH:
                    c0 = (c + 1) * CH
                    for do in range(ND):
                        eng = nc.sync if do % 2 == 0 else nc.gpsimd
                        eng.dma_start(
                            XT[:, do, c0 : c0 + CH], xt_r[:, do, c0 : c0 + CH]
                        )

                # ---- Q'T for this chunk: [d'-blk, q], contract d
                QTc = qtp.tile([P, ND, CH], MDT, tag="qt")
                for eb in range(ND):
                    pq = psQKV.tile([P, CH], F32, tag="qkv")
                    for do in range(ND):
                        nc.tensor.matmul(
                            pq,
                            A_sb[:, do, eb * P : (eb + 1) * P],
                            XT[:, do, c * CH : (c + 1) * CH],
                            start=(do == 0),
                            stop=(do == ND - 1),
                        )
                    nc.vector.tensor_copy(QTc[:, eb, :], pq)

                # ---- V for this chunk (chunk 0 emitted in prologue)
                if c > 0:
                    emit_v_chunk(c)

                # ---- scores^T + exp (+ causal mask on partial blocks)
                # For diagonal blocks (kb = 4c+i, i>0) only q-cols >= i*128
                # are causally live and AV never reads the dead columns, so
                # narrow the matmul/exp/mask to the live width (min 256 to
                # stay on the f32r full-rate path).
                nkb = QPC * (c + 1)
                for kb in range(nkb):
                    i = kb - QPC * c
                    q0 = max(i, 0) * P
                    if CH - q0 < 256:
                        q0 = CH - 256
                    W = CH - q0
                    ps_s = psA.tile([P, CH], F32, tag="a")
                    for eo in range(ND):
                        nc.tensor.matmul(
                            ps_s[:, 0:W],
                            XT[:, eo, kb * P : (kb + 1) * P],
                            QTc[:, eo, q0:CH],
                            start=(eo == 0),
                            stop=(eo == ND - 1),
                        )
                    ptw = {13: 384, 14: 256, 15: 256}.get(kb, CH)
                    base = CH - ptw
                    pt = ptp.tile([P, ptw], BF16, tag=f"pt{kb}")
                    nc.scalar.activation(
                        pt[:, q0 - base : CH - base],
                        ps_s[:, 0:W],
                        mybir.ActivationFunctionType.Exp,
                        scale=SCALE,
                    )
                    pt_tiles[kb] = (pt, base)
                    if kb >= QPC * c:
                        off = c * CH - kb * P + 384
                        nc.vector.tensor_mul(
                            pt[:, q0 - base : CH - base],
                            pt[:, q0 - base : CH - base],
                            smask[:, off + q0 - 256 : off + CH - 256],
                        )

                # ---- attn @ [V | 1], normalize, store
                for qs in range(QPC):
                    qb = c * QPC + qs
                    po0 = psO.tile([P, CH], F32, tag="o")
                    po1 = psO.tile([P, CH], F32, tag="o")
                    for kb in range(qb + 1):
                        ptk, pbase = pt_tiles[kb]
                        lhs = ptk[:, qs * P - pbase : (qs + 1) * P - pbase]
                        nc.tensor.matmul(
                            po0,
                            lhs,
                            V[:, kb, 0:CH],
                            start=(kb == 0),
                            stop=(kb == qb),
                        )
                        nc.tensor.matmul(
                            po1[:, 0 : D + 2 - CH],
                            lhs,
                            V[:, kb, CH : D + 2],
                            start=(kb == 0),
                            stop=(kb == qb),
                        )
                    recip = rcp.tile([P, 1], F32, tag="rc")
                    nc.vector.reciprocal(recip, po1[:, D - CH : D - CH + 1])
                    o_sb = outp.tile([P, D], F32, tag="o")
                    nc.vector.tensor_scalar_mul(o_sb[:, 0:CH], po0, recip)
                    nc.vector.tensor_scalar_mul(
                        o_sb[:, CH:D], po1[:, 0 : D - CH], recip
                    )
                    nc.scalar.dma_start(out_d[qb * P : (qb + 1) * P, :], o_sb)

    nc.finalize()
    return nc


_NC_CACHE = None


def _get_nc():
    global _NC_CACHE
    if _NC_CACHE is None:
        _NC_CACHE = _build_nc()
    return _NC_CACHE


def run(inputs, trace=False):
    x = np.asarray(inputs["x"], dtype=np.float32)
    wq = np.asarray(inputs["wq"], dtype=np.float32)
    wk = np.asarray(inputs["wk"], dtype=np.float32)
    wv = np.asarray(inputs["wv"], dtype=np.float32)
    nc = _get_nc()
    wqt = np.ascontiguousarray(wq.T)
    wkt = np.ascontiguousarray(wk.T)
    in_maps = [
        {
            "xt": np.ascontiguousarray(x[b].T),
            "wqt": wqt,
            "wkt": wkt,
            "wv": wv,
        }
        for b in range(B)
    ]
    res = run_bass_kernel_spmd(nc, in_maps, core_ids=list(range(B)), trace=trace)
    out = np.stack([r["out"] for r in res.results]).astype(np.float32)
    return out, res


def kernel(x, wq, wk, wv):
    out, _ = run({"x": x, "wq": wq, "wk": wk, "wv": wv}, trace=False)
    return out


# revision 9
# speedup vs baseline: 1.3194x; 1.0797x over previous
"""Causal attention (B=8, S=2048, D=768, single head) on 8 trn2 NeuronCores.

Sharding: data-parallel over batch — core b computes batch element b.

Host-side prep (layout/dtype only): x is passed transposed per core
(xt = x[b].T, [D, S]) and the weights are passed transposed (wqt = wq.T,
wkt = wk.T) — all cast to bf16 — so the kernel needs no PE transposes
and half the input DMA.

Algorithm per core:
  0. A = Wq @ Wk^T  [d, d'] computed once on device:
     A_blk = matmul(lhsT=wqt[e, d-blk], rhs=wkt[e, d']) contracting e.
     Then scores = x A x^T, so the K projection disappears:
     scoresT tile [k, q] = matmul(lhsT=xT[d, k-blk], rhs=Q'T[d, q]) with
     Q'T = matmul(lhsT=A[d, d'-blk], rhs=xT[d, s]) (Q' = x @ A).
  Per 512-wide q-chunk:
  1. Q'T chunk [d', q] (contract d over 6 blocks)
  2. V chunk [s, e] natural (lhsT = xT s-block, rhs = wv) + ones cols
  3. scoresT tiles [k-blk, q] contracting d, narrowed to the causally
     live q-range on diagonal blocks; exp (scale fused) on ACT -> P^T
     tiles; causal mask multiply on partial diagonal blocks
  4. out[q, e] (+ denominator via V ones cols) = PT_blk.T @ V_blk
     (contract k)
  5. normalize by reciprocal of denominator column, DMA out

All matmul operands are bf16 (full-rate PE, fp32 PSUM accumulation);
softmax weights P^T and V are bf16 (errors partially cancel through the
shared denominator).
"""

import numpy as np
import ml_dtypes

import concourse.bass as bass
import concourse.mybir as mybir
from concourse import bacc
from concourse.tile import TileContext
from concourse.bass_utils import run_bass_kernel_spmd

B, S, D = 8, 2048, 768
P = 128
ND = D // P            # 6 feature blocks
NB = S // P            # 16 seq blocks
CH = 512               # s-chunk width
NCH = S // CH          # 4 chunks
QPC = CH // P          # 4 q-blocks per chunk
SCALE = 1.0 / float(np.sqrt(D))
F32 = mybir.dt.float32
BF16 = mybir.dt.bfloat16


def _build_nc():
    nc = bacc.Bacc(None, target_bir_lowering=False)
    xt_d = nc.dram_tensor("xt", [D, S], BF16, kind="ExternalInput")
    wqt_d = nc.dram_tensor("wqt", [D, D], BF16, kind="ExternalInput")
    wkt_d = nc.dram_tensor("wkt", [D, D], BF16, kind="ExternalInput")
    wv_d = nc.dram_tensor("wv", [D, D], BF16, kind="ExternalInput")
    out_d = nc.dram_tensor("out", [S, D], F32, kind="ExternalOutput")

    xt_r = xt_d[:, :].rearrange("(o p) s -> p o s", p=P)
    wqt_r = wqt_d[:, :].rearrange("(o p) d -> p o d", p=P)
    wkt_r = wkt_d[:, :].rearrange("(o p) d -> p o d", p=P)
    wv_r = wv_d[:, :].rearrange("(o p) e -> p o e", p=P)

    with TileContext(nc) as tc:
        with (
            tc.tile_pool(name="const", bufs=1) as constp,
            tc.tile_pool(name="persist", bufs=1) as persist,
            tc.tile_pool(name="qt", bufs=1) as qtp,
            tc.tile_pool(name="pt", bufs=1) as ptp,
            tc.tile_pool(name="outp", bufs=2) as outp,
            tc.tile_pool(name="rc", bufs=4) as rcp,
            tc.tile_pool(name="psA", bufs=2, space="PSUM") as psA,
            tc.tile_pool(name="psQKV", bufs=2, space="PSUM") as psQKV,
            tc.tile_pool(name="psO", bufs=4, space="PSUM") as psO,
        ):
            # smask[p, g] = 1.0 if g >= p + 128 else 0.0 (bf16); slices give
            # the partial-diagonal causal masks for P^T tiles.
            smask_f = constp.tile([P, 640], F32)
            nc.gpsimd.memset(smask_f, 1.0)
            nc.gpsimd.affine_select(
                out=smask_f,
                in_=smask_f,
                compare_op=mybir.AluOpType.is_ge,
                fill=0.0,
                base=-128,
                pattern=[[1, 640]],
                channel_multiplier=-1,
            )
            smask = constp.tile([P, 640], BF16)
            nc.vector.tensor_copy(smask, smask_f)

            XT = persist.tile([P, ND, S], BF16)      # x^T: [d_in, do, s]
            V = persist.tile([P, NB, D + 2], BF16)   # [s_in, sb, e]; cols D,D+1 = 1.0
            WV = persist.tile([P, ND, D], BF16)      # wv: [d_in, do, e]
            A_sb = persist.tile([P, ND, D], BF16)    # A=wq@wk.T: [d_in, do, d']
            WQT = persist.tile([P, ND, D], BF16)     # wq^T: [e_in, eo, d]
            WKT = persist.tile([P, ND, D], BF16)     # wk^T: [e_in, eo, d']
            ones_col = constp.tile([P, NB, 2], F32)
            nc.vector.memset(ones_col, 1.0)
            nc.vector.tensor_copy(V[:, :, D : D + 2], ones_col)

            h3 = ND // 2

            # ---- prologue DMAs (first-needed first, HW queues for the
            # early pieces, software gpsimd queue for the c1 prefetch)
            for do in range(ND):
                nc.scalar.dma_start(XT[:, do, 0:CH], xt_r[:, do, 0:CH])
            nc.sync.dma_start(WV[:, 0:h3], wv_r[:, 0:h3])
            nc.sync.dma_start(WV[:, h3:ND], wv_r[:, h3:ND])
            nc.sync.dma_start(WQT[:, 0:ND], wqt_r[:, 0:ND])
            nc.sync.dma_start(WKT[:, 0:ND], wkt_r[:, 0:ND])

            # ---- PE warmup: dense dummy matmuls while prologue DMAs land.
            # Gets the HAM clock gate to K=8/8 (~3.4us of sustained PE
            # activity) before the first real matmul, instead of paying the
            # half-rate ramp on real work.
            warm_f = constp.tile([P, CH], F32)
            nc.vector.memset(warm_f, 0.0)
            warm = constp.tile([P, CH], BF16)
            nc.vector.tensor_copy(warm, warm_f)
            for w in range(14):
                pw = psA.tile([P, CH], F32, tag="a")
                nc.tensor.matmul(
                    pw, warm[:, 0:P], warm, start=True, stop=True
                )

            pt_tiles = {}

            def emit_v_chunk(c):
                for sb4 in range(QPC):
                    sb = c * QPC + sb4
                    s0 = sb * P
                    pv0 = psQKV.tile([P, CH], F32, tag="qkv")
                    for do in range(ND):
                        nc.tensor.matmul(
                            pv0,
                            XT[:, do, s0 : s0 + P],
                            WV[:, do, 0:CH],
                            start=(do == 0),
                            stop=(do == ND - 1),
                        )
                    nc.scalar.copy(V[:, sb, 0:CH], pv0)
                    pv1 = psQKV.tile([P, CH], F32, tag="qkv")
                    for do in range(ND):
                        nc.tensor.matmul(
                            pv1[:, 0 : D - CH],
                            XT[:, do, s0 : s0 + P],
                            WV[:, do, CH:D],
                            start=(do == 0),
                            stop=(do == ND - 1),
                        )
                    nc.scalar.copy(V[:, sb, CH:D], pv1[:, 0 : D - CH])

            # V(chunk 0) is the PE warmup work while wqt/wkt stream in.
            emit_v_chunk(0)

            # ---- A = Wq @ Wk^T, contracting e
            for db in range(ND):
                pa0 = psQKV.tile([P, CH], F32, tag="qkv")
                for eo in range(ND):
                    nc.tensor.matmul(
                        pa0,
                        WQT[:, eo, db * P : (db + 1) * P],
                        WKT[:, eo, 0:CH],
                        start=(eo == 0),
                        stop=(eo == ND - 1),
                    )
                nc.vector.tensor_copy(A_sb[:, db, 0:CH], pa0)
                pa1 = psQKV.tile([P, CH], F32, tag="qkv")
                for eo in range(ND):
                    nc.tensor.matmul(
                        pa1[:, 0 : D - CH],
                        WQT[:, eo, db * P : (db + 1) * P],
                        WKT[:, eo, CH:D],
                        start=(eo == 0),
                        stop=(eo == ND - 1),
                    )
                nc.vector.tensor_copy(A_sb[:, db, CH:D], pa1[:, 0 : D - CH])

            for c in range(NCH):
                # ---- prefetch next x chunk
                if c + 1 < NCH:
                    c0 = (c + 1) * CH
                    for do in range(ND):
                        eng = nc.sync if do % 2 == 0 else nc.gpsimd
                        eng.dma_start(
                            XT[:, do, c0 : c0 + CH], xt_r[:, do, c0 : c0 + CH]
                        )

                # ---- Q'T for this chunk: [d'-blk, q], contract d
                QTc = qtp.tile([P, ND, CH], BF16, tag="qt")
                for eb in range(ND):
                    pq = psQKV.tile([P, CH], F32, tag="qkv")
                    for do in range(ND):
                        nc.tensor.matmul(
                            pq,
                            A_sb[:, do, eb * P : (eb + 1) * P],
                            XT[:, do, c * CH : (c + 1) * CH],
                            start=(do == 0),
                            stop=(do == ND - 1),
                        )
                    nc.vector.tensor_copy(QTc[:, eb, :], pq)

                # ---- V for this chunk (chunk 0 emitted in prologue)
                if c > 0:
                    emit_v_chunk(c)

                # ---- scores^T + exp (+ causal mask on partial blocks)
                # For diagonal blocks (kb = 4c+i) only q-cols >= i*128 are
                # causally live and AV never reads the dead columns, so
                # narrow the matmul/exp/mask to the live width.
                nkb = QPC * (c + 1)
                for kb in range(nkb):
                    i = kb - QPC * c
                    q0 = max(i, 0) * P
                    W = CH - q0
                    ps_s = psA.tile([P, CH], F32, tag="a")
                    for eo in range(ND):
                        nc.tensor.matmul(
                            ps_s[:, 0:W],
                            XT[:, eo, kb * P : (kb + 1) * P],
                            QTc[:, eo, q0:CH],
                            start=(eo == 0),
                            stop=(eo == ND - 1),
                        )
                    ptw = {13: 384, 14: 256, 15: 128}.get(kb, CH)
                    base = CH - ptw
                    pt = ptp.tile([P, ptw], BF16, tag=f"pt{kb}")
                    nc.scalar.activation(
                        pt[:, q0 - base : CH - base],
                        ps_s[:, 0:W],
                        mybir.ActivationFunctionType.Exp,
                        scale=SCALE,
                    )
                    pt_tiles[kb] = (pt, base)
                    if kb >= QPC * c:
                        off = c * CH - kb * P + 384
                        nc.vector.tensor_mul(
                            pt[:, q0 - base : CH - base],
                            pt[:, q0 - base : CH - base],
                            smask[:, off + q0 - 256 : off + CH - 256],
                        )

                # ---- attn @ [V | 1], normalize, store
                for qs in range(QPC):
                    qb = c * QPC + qs
                    po0 = psO.tile([P, CH], F32, tag="o")
                    po1 = psO.tile([P, CH], F32, tag="o")
                    for kb in range(qb + 1):
                        ptk, pbase = pt_tiles[kb]
                        lhs = ptk[:, qs * P - pbase : (qs + 1) * P - pbase]
                        nc.tensor.matmul(
                            po0,
                            lhs,
                            V[:, kb, 0:CH],
                            start=(kb == 0),
                            stop=(kb == qb),
                        )
                        nc.tensor.matmul(
                            po1[:, 0 : D + 2 - CH],
                            lhs,
                            V[:, kb, CH : D + 2],
                            start=(kb == 0),
                            stop=(kb == qb),
                        )
                    recip = rcp.tile([P, 1], F32, tag="rc")
                    nc.vector.reciprocal(recip, po1[:, D - CH : D - CH + 1])
                    o_sb = outp.tile([P, D], F32, tag="o")
                    nc.vector.tensor_scalar_mul(o_sb[:, 0:CH], po0, recip)
                    nc.vector.tensor_scalar_mul(
                        o_sb[:, CH:D], po1[:, 0 : D - CH], recip
                    )
                    nc.scalar.dma_start(out_d[qb * P : (qb + 1) * P, :], o_sb)

    nc.finalize()
    return nc


_NC_CACHE = None


def _get_nc():
    global _NC_CACHE
    if _NC_CACHE is None:
        _NC_CACHE = _build_nc()
    return _NC_CACHE


def run(inputs, trace=False):
    x = np.asarray(inputs["x"], dtype=np.float32)
    wq = np.asarray(inputs["wq"], dtype=np.float32)
    wk = np.asarray(inputs["wk"], dtype=np.float32)
    wv = np.asarray(inputs["wv"], dtype=np.float32)
    nc = _get_nc()
    bf = ml_dtypes.bfloat16
    wqt = np.ascontiguousarray(wq.T).astype(bf)
    wkt = np.ascontiguousarray(wk.T).astype(bf)
    wv_b = wv.astype(bf)
    in_maps = [
        {
            "xt": np.ascontiguousarray(x[b].T).astype(bf),
            "wqt": wqt,
            "wkt": wkt,
            "wv": wv_b,
        }
        for b in range(B)
    ]
    res = run_bass_kernel_spmd(nc, in_maps, core_ids=list(range(B)), trace=trace)
    out = np.stack([r["out"] for r in res.results]).astype(np.float32)
    return out, res


def kernel(x, wq, wk, wv):
    out, _ = run({"x": x, "wq": wq, "wk": wk, "wv": wv}, trace=False)
    return out
